# revision 1
# baseline (speedup 1.0000x reference)
"""Trainium2 Bass kernel for column-softmax attention.

reference semantics:
    scores = einsum('bqd,bkd->bqk', q, k) / sqrt(128)   # [B, Nq, Nk]
    attn   = softmax(scores, axis=1)                     # over the QUERY axis
    out    = einsum('bqk,bkd->bqd', attn, v)             # [B, Nq, D]

Because the softmax is over q, each key column k normalizes independently:
    out[q, d] = sum_k E[k, q] * r[k] * v[k, d],  E = exp(scores.T), r = 1/sum_q E[k, q]

Sharding: 8 cores = 4 batches x 2 key-halves.  Each core computes the partial
sum over its 2048 keys; the host adds the two partials per batch.

On-chip layout: the host pre-transposes Q and K to [D, N] (contraction dim on
partitions) and the kernel emits out.T [D, Nq]; the host transposes back.  The
softmax denominator is folded into V row-scaling so the normalize step touches
only 128x128 tiles per key tile.

Phase A (per key tile): scores matmul (fp16 in, fp32 psum) -> exp on ScalarE
(row-sums half fused into the activation, half on VectorE) -> E resident in
SBUF as fp16 -> this key tile's contribution to out.T for query half A
accumulated in PSUM (hides the second GEMM under the ScalarE exp span).
Phase B: query half B accumulated over all 16 key tiles, flushed, DMA'd out.

The ScalarE exp pass (8.4M elements/core, ~55us) is the roofline; measured
body time ~85-90us via the For_i loop-differencing method (see test.py).  PE
weight-load transitions cost ~1us each on this toolchain, so matmuls are
ordered to maximize consecutive same-stationary runs (explicit ordering deps
keep the scheduler from splitting them).
"""

import numpy as np

import concourse.bass as bass
import concourse.mybir as mybir
import concourse.tile as tile
from concourse.bass_utils import run_bass_kernel_spmd
from concourse.tile_rust import add_dep_helper

B, N, D = 4, 4096, 128
P = 128
NK = 2048                 # keys per core (half of 4096)
KT_TILES = NK // P        # 16 key tiles of 128
SCALE = 1.0 / np.sqrt(128.0)

F32 = mybir.dt.float32
F16 = mybir.dt.float16


def emit_body(nc, tc, pools, aps, skip_act=False, skip_phaseb=False, skip_gemm1=False, c2048=False, dve_rowsum=False):
    big, epool, small, spsum, opsum = pools
    qt_d, kt_d, v_d, out_d = aps

    qT = big.tile([P, N], F16, tag="qT")            # [d, q]
    kT = big.tile([P, NK], F16, tag="kT")           # [d, k]
    vsb = big.tile([P, KT_TILES, D], F16, tag="v")  # [k_in_tile, k_tile, d]
    oacc = big.tile([P, N], F32, tag="oacc")        # [d, q]

    for _qc in range(4):
        nc.sync.dma_start(
            qT[:, _qc * 1024 : (_qc + 1) * 1024], qt_d[:, _qc * 1024 : (_qc + 1) * 1024]
        )
    nc.sync.dma_start(kT[:], kt_d[:])
    nc.sync.dma_start(vsb[:], v_d.rearrange("(t p) d -> p t d", p=P))

    s_free = 2048 if c2048 else 1024
    # Warm-up matmul: first real matmul then carries at most one sync wait.
    Swarm = spsum.tile([P, s_free], F32, tag="S")
    nc.tensor.matmul(
        Swarm[0:1, 0:1], lhsT=kT[:, 0:1], rhs=qT[:, 0:1], start=True, stop=True
    )

    e_tiles = []
    v_tiles = []
    # Output accumulators for query half A (cols 0..2047) are built up during
    # phase A so most of the second GEMM hides under the exp (ScalarE) span.
    oa_tiles = []
    if not skip_phaseb and not c2048:
        for _oc in range(2):
            O_a = opsum.tile([P, 1024], F32, tag="O")
            oa_tiles.append(O_a)

    # Phase A: per key tile, scores + exp (row-sum fused) + scaled V,
    # then this key tile's contribution to out.T for query half A.
    for kt in range(KT_TILES):
        E = epool.tile([P, N], F16, tag=f"E{kt}")   # [k, q] = exp(scores.T)
        n_h = N // s_free
        rs = small.tile([P, n_h], F32, tag=f"rs{kt}")
        last_g1 = None
        for h in range(0 if skip_gemm1 else n_h):
            S = spsum.tile([P, s_free], F32, tag="S")
            for u in range(s_free // 512):
                last_g1 = nc.tensor.matmul(
                    S[:, u * 512 : (u + 1) * 512],
                    lhsT=kT[:, kt * P : (kt + 1) * P],
                    rhs=qT[:, h * s_free + u * 512 : h * s_free + u * 512 + 512],
                    start=True,
                    stop=True,
                )
            if not skip_act:
                if dve_rowsum and h < n_h // 2:
                    # row-sum for this chunk computed on VectorE from E
                    # (frees the ScalarE accumulator-read overhead)
                    nc.scalar.activation(
                        out=E[:, h * s_free : (h + 1) * s_free],
                        in_=S[:],
                        func=mybir.ActivationFunctionType.Exp,
                        scale=float(SCALE),
                    )
                    nc.vector.reduce_sum(
                        out=rs[:, h : h + 1],
                        in_=E[:, h * s_free : (h + 1) * s_free],
                        axis=mybir.AxisListType.X,
                    )
                else:
                    nc.scalar.activation(
                        out=E[:, h * s_free : (h + 1) * s_free],
                        in_=S[:],
                        func=mybir.ActivationFunctionType.Exp,
                        scale=float(SCALE),
                        accum_out=rs[:, h : h + 1],
                    )
        rsum = small.tile([P, 1], F32, tag="rsum")
        recip = small.tile([P, 1], F32, tag="recip")
        vsc = small.tile([P, D], F16, tag=f"vsc{kt}")  # [k, d] * r[k]
        if not skip_act:
            nc.vector.reduce_sum(out=rsum[:], in_=rs[:], axis=mybir.AxisListType.X)
            nc.vector.reciprocal(recip[:], rsum[:])
            nc.vector.tensor_scalar_mul(vsc[:], vsb[:, kt, :], recip[:])
        elif not skip_phaseb:
            nc.sync.dma_start(E[:], qt_d[:])
            nc.vector.tensor_copy(out=vsc[:], in_=vsb[:, kt, :])
        e_tiles.append(E)
        v_tiles.append(vsc)

        if not skip_phaseb and not c2048:
            # Emit the PREVIOUS key tile's half-A output matmuls here, ordered
            # after this tile's scores matmuls (ordering-only deps).  This
            # keeps each kT weight-load run contiguous: the scheduler would
            # otherwise wedge the vsc burst into the middle of the scores
            # run, costing an extra ~1us weight transition per key tile.
            if kt > 0:
                pv, pe_t, pkt = pending_g2a
                for oc in range(2):
                    for u in range(2):
                        mm = nc.tensor.matmul(
                            oa_tiles[oc][:, u * 512 : (u + 1) * 512],
                            lhsT=pv[:],
                            rhs=pe_t[:, oc * 1024 + u * 512 : oc * 1024 + (u + 1) * 512],
                            start=(pkt == 0),
                            stop=False,
                        )
                        if last_g1 is not None:
                            add_dep_helper(
                                mm.ins,
                                last_g1.ins,
                                sync=False,
                                reason="keep kT weight-load run contiguous",
                            )
            pending_g2a = (vsc, E, kt)

    if not skip_phaseb and not c2048:
        pv, pe_t, pkt = pending_g2a
        for oc in range(2):
            for u in range(2):
                nc.tensor.matmul(
                    oa_tiles[oc][:, u * 512 : (u + 1) * 512],
                    lhsT=pv[:],
                    rhs=pe_t[:, oc * 1024 + u * 512 : oc * 1024 + (u + 1) * 512],
                    start=False,
                    stop=True,
                )

    # Phase B: flush half A, then accumulate query half B (cols 2048..4095).
    if not skip_phaseb and c2048:
        # no interleave: both halves accumulated here, kt-outer.
        # O tiles live in the S pool slots (the opsum pool has no banks
        # left when S is [128, 2048] double-buffered).
        for half in range(2):
            hb_tiles = []
            for _oc in range(2):
                O_h = spsum.tile([P, 1024], F32, tag="S")
                hb_tiles.append(O_h)
            for kt in range(KT_TILES):
                for oc in range(2):
                    for u in range(2):
                        base = half * 2048 + oc * 1024 + u * 512
                        nc.tensor.matmul(
                            hb_tiles[oc][:, u * 512 : (u + 1) * 512],
                            lhsT=v_tiles[kt][:],
                            rhs=e_tiles[kt][:, base : base + 512],
                            start=(kt == 0),
                            stop=(kt == KT_TILES - 1),
                        )
            for oc in range(2):
                lo = half * 2048 + oc * 1024
                nc.vector.tensor_copy(out=oacc[:, lo : lo + 1024], in_=hb_tiles[oc][:])
                nc.sync.dma_start(out_d[:, lo : lo + 1024], oacc[:, lo : lo + 1024])
    elif not skip_phaseb:
        for oc in range(2):
            nc.vector.tensor_copy(
                out=oacc[:, oc * 1024 : (oc + 1) * 1024], in_=oa_tiles[oc][:]
            )
            nc.sync.dma_start(
                out_d[:, oc * 1024 : (oc + 1) * 1024],
                oacc[:, oc * 1024 : (oc + 1) * 1024],
            )
        ob_tiles = []
        for _oc in range(2):
            O_b = opsum.tile([P, 1024], F32, tag="O")
            ob_tiles.append(O_b)
        prev_mm = None
        for kt in range(KT_TILES):
            for oc in range(2):
                for u in range(2):
                    mm = nc.tensor.matmul(
                        ob_tiles[oc][:, u * 512 : (u + 1) * 512],
                        lhsT=v_tiles[kt][:],
                        rhs=e_tiles[kt][:, 2048 + oc * 1024 + u * 512 : 2048 + oc * 1024 + (u + 1) * 512],
                        start=(kt == 0),
                        stop=(kt == KT_TILES - 1),
                    )
                    # chain ordering so each vsc weight-load run stays a
                    # contiguous block of 4 (scheduler otherwise splits the
                    # first few key tiles into 2+2 across the O tiles)
                    if prev_mm is not None:
                        add_dep_helper(
                            mm.ins, prev_mm.ins, sync=False,
                            reason="contiguous vsc weight runs in tail",
                        )
                    prev_mm = mm
        for oc in range(2):
            nc.vector.tensor_copy(
                out=oacc[:, 2048 + oc * 1024 : 2048 + (oc + 1) * 1024],
                in_=ob_tiles[oc][:],
            )
            nc.sync.dma_start(
                out_d[:, 2048 + oc * 1024 : 2048 + (oc + 1) * 1024],
                oacc[:, 2048 + oc * 1024 : 2048 + (oc + 1) * 1024],
            )
    else:
        nc.gpsimd.memset(oacc[:], 0.0)
        nc.sync.dma_start(out_d[:], oacc[:])


def build_bass(repeat=1, skip_act=False, skip_phaseb=False, skip_gemm1=False, loop=False, c2048=False, dve_rowsum=False):
    nc = bass.Bass("TRN2", target_bir_lowering=False, debug=False)
    qt_d = nc.dram_tensor("qt", [P, N], F16, kind="ExternalInput").ap()
    kt_d = nc.dram_tensor("kt", [P, NK], F16, kind="ExternalInput").ap()
    v_d = nc.dram_tensor("v", [NK, D], F16, kind="ExternalInput").ap()
    out_d = nc.dram_tensor("out_t", [P, N], F32, kind="ExternalOutput").ap()

    with tile.TileContext(nc) as tc:
        import contextlib
        with (
            tc.tile_pool(name="big", bufs=1) as big,
            tc.tile_pool(name="epool", bufs=1) as epool,
            tc.tile_pool(name="small", bufs=2) as small,
            tc.tile_pool(name="spsum", bufs=2, space="PSUM") as spsum,
            (contextlib.nullcontext(None) if c2048
             else tc.tile_pool(name="opsum", bufs=2, space="PSUM")) as opsum,
        ):
            def body():
                emit_body(
                    nc,
                    tc,
                    (big, epool, small, spsum, opsum),
                    (qt_d, kt_d, v_d, out_d),
                    skip_act=skip_act,
                    skip_phaseb=skip_phaseb,
                    skip_gemm1=skip_gemm1,
                    c2048=c2048,
                    dve_rowsum=dve_rowsum,
                )

            if loop and repeat > 1:
                with tc.For_i(
                    0, repeat, 1,
                    hint_engines=(mybir.EngineType.PE, mybir.EngineType.Activation),
                ):
                    body()
            else:
                for _ in range(repeat):
                    body()
    return nc


def legalize_waits(nc, max_waits=1):
    """Hoist excess semaphore waits into standalone EventSemaphore ops.

    The walrus codegen for several engine instruction structs accepts only a
    single sync-wait command; Tile sometimes emits more.  Executing the extra
    waits in a preceding same-engine EventSemaphore is semantically identical
    (the engine runs its stream in order).
    """
    for fn in nc.m.functions:
        for blk in fn.blocks:
            out = []
            for inst in blk.instructions:
                si = inst.sync_info
                if (
                    si is not None
                    and si.on_wait
                    and len(si.on_wait) > max_waits
                    and inst.opcode != "EventSemaphore"
                ):
                    waits = list(si.on_wait)
                    extra, keep = waits[:-max_waits], waits[-max_waits:]
                    for n, w in enumerate(extra):
                        out.append(
                            mybir.InstEventSemaphore(
                                name=f"{inst.name}_prewait{n}",
                                engine=inst.engine,
                                ins=[],
                                outs=[],
                                sync_info=mybir.SyncInfo(on_wait=[w], on_update=[]),
                            )
                        )
                    si.on_wait = keep
                out.append(inst)
            blk.instructions = out
    return nc


_NC_CACHE = {}


def _get_nc(repeat=1, **kw):
    key = ("nc", repeat, tuple(sorted(kw.items())))
    if key not in _NC_CACHE:
        _NC_CACHE[key] = legalize_waits(build_bass(repeat, **kw))
    return _NC_CACHE[key]


def kernel(q, k, v):
    q = np.asarray(q, dtype=np.float32)
    k = np.asarray(k, dtype=np.float32)
    v = np.asarray(v, dtype=np.float32)

    in_maps = []
    for c in range(8):
        b, h = c // 2, c % 2
        in_maps.append(
            {
                "qt": np.ascontiguousarray(q[b].T).astype(np.float16),
                "kt": np.ascontiguousarray(k[b, h * NK : (h + 1) * NK].T).astype(np.float16),
                "v": np.ascontiguousarray(v[b, h * NK : (h + 1) * NK]).astype(np.float16),
            }
        )

    nc = _get_nc()
    res = run_bass_kernel_spmd(nc, in_maps, list(range(8))).results

    out = np.empty((B, N, D), dtype=np.float32)
    for b in range(B):
        out[b] = (res[2 * b]["out_t"] + res[2 * b + 1]["out_t"]).T
    return out



# revision 2
# speedup vs baseline: 1.3757x; 1.3757x over previous
"""Trainium2 Bass kernel for column-softmax attention.

reference semantics:
    scores = einsum('bqd,bkd->bqk', q, k) / sqrt(128)   # [B, Nq, Nk]
    attn   = softmax(scores, axis=1)                     # over the QUERY axis
    out    = einsum('bqk,bkd->bqd', attn, v)             # [B, Nq, D]

Because the softmax is over q, each key column k normalizes independently:
    out[q, d] = sum_k E[k, q] * r[k] * v[k, d],  E = exp(scores.T), r = 1/sum_q E[k, q]

Sharding: 8 cores = 4 batches x 2 key-halves.  Each core computes the partial
sum over its 2048 keys; the host adds the two partials per batch.

On-chip layout: the host pre-transposes Q and K to [D, N] (contraction dim on
partitions) and the kernel emits out.T [D, Nq]; the host transposes back.  The
softmax denominator is folded into V row-scaling so the normalize step touches
only 128x128 tiles per key tile.

Phase A (per key tile): scores matmul (fp16 in, fp32 psum) -> exp on ScalarE
(row-sums half fused into the activation, half on VectorE) -> E resident in
SBUF as fp16 -> this key tile's contribution to out.T for query half A
accumulated in PSUM (hides the second GEMM under the ScalarE exp span).
Phase B: query half B accumulated over all 16 key tiles, flushed, DMA'd out.

The ScalarE exp pass (8.4M elements/core) is the roofline.  The first
score matmul only needs the first key tile and first q chunk, so those land
in their own SBUF tiles with their DMAs issued first (the Tile dep tracker
is whole-tile: one big qT tile would stall the first matmul on all four qT
piece DMAs, ~4us of dead startup).  Matmuls are ordered to keep stationary
runs contiguous (explicit ordering deps keep the scheduler from splitting
them); paired A/B loop-diff timing on hardware measured this layout ~20%
faster than the single-tile baseline.
"""

import numpy as np

import concourse.bass as bass
import concourse.mybir as mybir
import concourse.tile as tile
from concourse.bass_utils import run_bass_kernel_spmd
from concourse.tile_rust import add_dep_helper

B, N, D = 4, 4096, 128
P = 128
NK = 2048                 # keys per core (half of 4096)
KT_TILES = NK // P        # 16 key tiles of 128
SCALE = 1.0 / np.sqrt(128.0)

F32 = mybir.dt.float32
F16 = mybir.dt.float16


def emit_body(nc, tc, pools, aps, skip_act=False, skip_phaseb=False, skip_gemm1=False, c2048=False, dve_rowsum=False):
    big, epool, small, spsum, opsum = pools
    qt_d, kt_d, v_d, out_d = aps

    qT0 = big.tile([P, 1024], F16, tag="qT0")       # [d, q 0:1024]
    qTr = big.tile([P, N - 1024], F16, tag="qTr")   # [d, q 1024:4096]
    kT0 = big.tile([P, P], F16, tag="kT0")          # [d, k tile 0]
    kTr = big.tile([P, NK - P], F16, tag="kTr")     # [d, k tiles 1..15]
    vsb = big.tile([P, KT_TILES, D], F16, tag="v")  # [k_in_tile, k_tile, d]
    oacc = big.tile([P, N], F32, tag="oacc")        # [d, q]

    def kslice(kt, lo=0, hi=P):
        if kt == 0:
            return kT0[:, lo:hi]
        return kTr[:, (kt - 1) * P + lo : (kt - 1) * P + hi]

    def qslice(lo, width):
        if lo + width <= 1024:
            return qT0[:, lo : lo + width]
        assert lo >= 1024
        return qTr[:, lo - 1024 : lo - 1024 + width]

    # first key tile + first q chunk land first so the pipeline starts early
    nc.sync.dma_start(kT0[:], kt_d[:, 0:P])
    nc.sync.dma_start(qT0[:], qt_d[:, 0:1024])
    nc.sync.dma_start(kTr[:], kt_d[:, P:NK])
    for _qc in range(1, 4):
        nc.sync.dma_start(
            qTr[:, (_qc - 1) * 1024 : _qc * 1024],
            qt_d[:, _qc * 1024 : (_qc + 1) * 1024],
        )
    nc.sync.dma_start(vsb[:], v_d.rearrange("(t p) d -> p t d", p=P))

    s_free = 2048 if c2048 else 1024
    # Warm-up matmul: first real matmul then carries at most one sync wait.
    Swarm = spsum.tile([P, s_free], F32, tag="S")
    nc.tensor.matmul(
        Swarm[0:1, 0:1], lhsT=kT0[:, 0:1], rhs=qT0[:, 0:1], start=True, stop=True
    )

    e_tiles = []
    v_tiles = []
    # Output accumulators for query half A (cols 0..2047) are built up during
    # phase A so most of the second GEMM hides under the exp (ScalarE) span.
    oa_tiles = []
    if not skip_phaseb and not c2048:
        for _oc in range(2):
            O_a = opsum.tile([P, 1024], F32, tag="O")
            oa_tiles.append(O_a)

    # Phase A: per key tile, scores + exp (row-sum fused) + scaled V,
    # then this key tile's contribution to out.T for query half A.
    for kt in range(KT_TILES):
        E = epool.tile([P, N], F16, tag=f"E{kt}")   # [k, q] = exp(scores.T)
        n_h = N // s_free
        rs = small.tile([P, n_h], F32, tag=f"rs{kt}")
        last_g1 = None
        for h in range(0 if skip_gemm1 else n_h):
            S = spsum.tile([P, s_free], F32, tag="S")
            for u in range(s_free // 512):
                last_g1 = nc.tensor.matmul(
                    S[:, u * 512 : (u + 1) * 512],
                    lhsT=kslice(kt),
                    rhs=qslice(h * s_free + u * 512, 512),
                    start=True,
                    stop=True,
                )
            if not skip_act:
                if dve_rowsum and h < n_h // 2:
                    # row-sum for this chunk computed on VectorE from E
                    # (frees the ScalarE accumulator-read overhead)
                    nc.scalar.activation(
                        out=E[:, h * s_free : (h + 1) * s_free],
                        in_=S[:],
                        func=mybir.ActivationFunctionType.Exp,
                        scale=float(SCALE),
                    )
                    nc.vector.reduce_sum(
                        out=rs[:, h : h + 1],
                        in_=E[:, h * s_free : (h + 1) * s_free],
                        axis=mybir.AxisListType.X,
                    )
                else:
                    nc.scalar.activation(
                        out=E[:, h * s_free : (h + 1) * s_free],
                        in_=S[:],
                        func=mybir.ActivationFunctionType.Exp,
                        scale=float(SCALE),
                        accum_out=rs[:, h : h + 1],
                    )
        rsum = small.tile([P, 1], F32, tag="rsum")
        recip = small.tile([P, 1], F32, tag="recip")
        vsc = small.tile([P, D], F16, tag=f"vsc{kt}")  # [k, d] * r[k]
        if not skip_act:
            nc.vector.reduce_sum(out=rsum[:], in_=rs[:], axis=mybir.AxisListType.X)
            nc.vector.reciprocal(recip[:], rsum[:])
            nc.vector.tensor_scalar_mul(vsc[:], vsb[:, kt, :], recip[:])
        elif not skip_phaseb:
            nc.sync.dma_start(E[:], qt_d[:])  # debug path
            nc.vector.tensor_copy(out=vsc[:], in_=vsb[:, kt, :])
        e_tiles.append(E)
        v_tiles.append(vsc)

        if not skip_phaseb and not c2048:
            # Emit the PREVIOUS key tile's half-A output matmuls here, ordered
            # after this tile's scores matmuls (ordering-only deps).  This
            # keeps each kT weight-load run contiguous: the scheduler would
            # otherwise wedge the vsc burst into the middle of the scores
            # run, costing an extra ~1us weight transition per key tile.
            if kt > 0:
                pv, pe_t, pkt = pending_g2a
                for oc in range(2):
                    for u in range(2):
                        mm = nc.tensor.matmul(
                            oa_tiles[oc][:, u * 512 : (u + 1) * 512],
                            lhsT=pv[:],
                            rhs=pe_t[:, oc * 1024 + u * 512 : oc * 1024 + (u + 1) * 512],
                            start=(pkt == 0),
                            stop=False,
                        )
                        if last_g1 is not None:
                            add_dep_helper(
                                mm.ins,
                                last_g1.ins,
                                sync=False,
                                reason="keep kT weight-load run contiguous",
                            )
            pending_g2a = (vsc, E, kt)

    if not skip_phaseb and not c2048:
        pv, pe_t, pkt = pending_g2a
        for oc in range(2):
            for u in range(2):
                nc.tensor.matmul(
                    oa_tiles[oc][:, u * 512 : (u + 1) * 512],
                    lhsT=pv[:],
                    rhs=pe_t[:, oc * 1024 + u * 512 : oc * 1024 + (u + 1) * 512],
                    start=False,
                    stop=True,
                )

    # Phase B: flush half A, then accumulate query half B (cols 2048..4095).
    if not skip_phaseb and c2048:
        # no interleave: both halves accumulated here, kt-outer.
        # O tiles live in the S pool slots (the opsum pool has no banks
        # left when S is [128, 2048] double-buffered).
        for half in range(2):
            hb_tiles = []
            for _oc in range(2):
                O_h = spsum.tile([P, 1024], F32, tag="S")
                hb_tiles.append(O_h)
            for kt in range(KT_TILES):
                for oc in range(2):
                    for u in range(2):
                        base = half * 2048 + oc * 1024 + u * 512
                        nc.tensor.matmul(
                            hb_tiles[oc][:, u * 512 : (u + 1) * 512],
                            lhsT=v_tiles[kt][:],
                            rhs=e_tiles[kt][:, base : base + 512],
                            start=(kt == 0),
                            stop=(kt == KT_TILES - 1),
                        )
            for oc in range(2):
                lo = half * 2048 + oc * 1024
                nc.vector.tensor_copy(out=oacc[:, lo : lo + 1024], in_=hb_tiles[oc][:])
                nc.sync.dma_start(out_d[:, lo : lo + 1024], oacc[:, lo : lo + 1024])
    elif not skip_phaseb:
        for oc in range(2):
            nc.vector.tensor_copy(
                out=oacc[:, oc * 1024 : (oc + 1) * 1024], in_=oa_tiles[oc][:]
            )
            nc.sync.dma_start(
                out_d[:, oc * 1024 : (oc + 1) * 1024],
                oacc[:, oc * 1024 : (oc + 1) * 1024],
            )
        ob_tiles = []
        for _oc in range(2):
            O_b = opsum.tile([P, 1024], F32, tag="O")
            ob_tiles.append(O_b)
        prev_mm = None
        for kt in range(KT_TILES):
            for oc in range(2):
                for u in range(2):
                    mm = nc.tensor.matmul(
                        ob_tiles[oc][:, u * 512 : (u + 1) * 512],
                        lhsT=v_tiles[kt][:],
                        rhs=e_tiles[kt][:, 2048 + oc * 1024 + u * 512 : 2048 + oc * 1024 + (u + 1) * 512],
                        start=(kt == 0),
                        stop=(kt == KT_TILES - 1),
                    )
                    # chain ordering so each vsc weight-load run stays a
                    # contiguous block of 4 (scheduler otherwise splits the
                    # first few key tiles into 2+2 across the O tiles)
                    if prev_mm is not None:
                        add_dep_helper(
                            mm.ins, prev_mm.ins, sync=False,
                            reason="contiguous vsc weight runs in tail",
                        )
                    prev_mm = mm
        for oc in range(2):
            nc.vector.tensor_copy(
                out=oacc[:, 2048 + oc * 1024 : 2048 + (oc + 1) * 1024],
                in_=ob_tiles[oc][:],
            )
            nc.sync.dma_start(
                out_d[:, 2048 + oc * 1024 : 2048 + (oc + 1) * 1024],
                oacc[:, 2048 + oc * 1024 : 2048 + (oc + 1) * 1024],
            )
    else:
        nc.gpsimd.memset(oacc[:], 0.0)
        nc.sync.dma_start(out_d[:], oacc[:])


def build_bass(repeat=1, skip_act=False, skip_phaseb=False, skip_gemm1=False, loop=False, c2048=False, dve_rowsum=False):
    nc = bass.Bass("TRN2", target_bir_lowering=False, debug=False)
    qt_d = nc.dram_tensor("qt", [P, N], F16, kind="ExternalInput").ap()
    kt_d = nc.dram_tensor("kt", [P, NK], F16, kind="ExternalInput").ap()
    v_d = nc.dram_tensor("v", [NK, D], F16, kind="ExternalInput").ap()
    out_d = nc.dram_tensor("out_t", [P, N], F32, kind="ExternalOutput").ap()

    with tile.TileContext(nc) as tc:
        import contextlib
        with (
            tc.tile_pool(name="big", bufs=1) as big,
            tc.tile_pool(name="epool", bufs=1) as epool,
            tc.tile_pool(name="small", bufs=2) as small,
            tc.tile_pool(name="spsum", bufs=2, space="PSUM") as spsum,
            (contextlib.nullcontext(None) if c2048
             else tc.tile_pool(name="opsum", bufs=2, space="PSUM")) as opsum,
        ):
            def body():
                emit_body(
                    nc,
                    tc,
                    (big, epool, small, spsum, opsum),
                    (qt_d, kt_d, v_d, out_d),
                    skip_act=skip_act,
                    skip_phaseb=skip_phaseb,
                    skip_gemm1=skip_gemm1,
                    c2048=c2048,
                    dve_rowsum=dve_rowsum,
                )

            if loop and repeat > 1:
                with tc.For_i(
                    0, repeat, 1,
                    hint_engines=(mybir.EngineType.PE, mybir.EngineType.Activation),
                ):
                    body()
            else:
                for _ in range(repeat):
                    body()
    return nc


def legalize_waits(nc, max_waits=1):
    """Hoist excess semaphore waits into standalone EventSemaphore ops.

    The walrus codegen for several engine instruction structs accepts only a
    single sync-wait command; Tile sometimes emits more.  Executing the extra
    waits in a preceding same-engine EventSemaphore is semantically identical
    (the engine runs its stream in order).
    """
    for fn in nc.m.functions:
        for blk in fn.blocks:
            out = []
            for inst in blk.instructions:
                si = inst.sync_info
                if (
                    si is not None
                    and si.on_wait
                    and len(si.on_wait) > max_waits
                    and inst.opcode != "EventSemaphore"
                ):
                    waits = list(si.on_wait)
                    extra, keep = waits[:-max_waits], waits[-max_waits:]
                    for n, w in enumerate(extra):
                        out.append(
                            mybir.InstEventSemaphore(
                                name=f"{inst.name}_prewait{n}",
                                engine=inst.engine,
                                ins=[],
                                outs=[],
                                sync_info=mybir.SyncInfo(on_wait=[w], on_update=[]),
                            )
                        )
                    si.on_wait = keep
                out.append(inst)
            blk.instructions = out
    return nc


_NC_CACHE = {}


def _get_nc(repeat=1, **kw):
    key = ("nc", repeat, tuple(sorted(kw.items())))
    if key not in _NC_CACHE:
        _NC_CACHE[key] = legalize_waits(build_bass(repeat, **kw))
    return _NC_CACHE[key]


def kernel(q, k, v):
    q = np.asarray(q, dtype=np.float32)
    k = np.asarray(k, dtype=np.float32)
    v = np.asarray(v, dtype=np.float32)

    in_maps = []
    for c in range(8):
        b, h = c // 2, c % 2
        in_maps.append(
            {
                "qt": np.ascontiguousarray(q[b].T).astype(np.float16),
                "kt": np.ascontiguousarray(k[b, h * NK : (h + 1) * NK].T).astype(np.float16),
                "v": np.ascontiguousarray(v[b, h * NK : (h + 1) * NK]).astype(np.float16),
            }
        )

    nc = _get_nc()
    res = run_bass_kernel_spmd(nc, in_maps, list(range(8))).results

    out = np.empty((B, N, D), dtype=np.float32)
    for b in range(B):
        out[b] = (res[2 * b]["out_t"] + res[2 * b + 1]["out_t"]).T
    return out



# revision 6
# speedup vs baseline: 1.8586x; 1.3510x over previous
"""Trainium2 Bass kernel for column-softmax attention.

reference semantics:
    scores = einsum('bqd,bkd->bqk', q, k) / sqrt(128)   # [B, Nq, Nk]
    attn   = softmax(scores, axis=1)                     # over the QUERY axis
    out    = einsum('bqk,bkd->bqd', attn, v)             # [B, Nq, D]

Because the softmax is over q, each key column k normalizes independently:
    out[q, d] = sum_k E[k, q] * r[k] * v[k, d],  E = exp(scores.T), r = 1/sum_q E[k, q]

Sharding: 8 cores = 4 batches x 2 key-halves.  Each core computes the partial
sum over its 2048 keys; the host adds the two partials per batch.

On-chip layout: the host pre-transposes Q and K to [D, N] (contraction dim on
partitions) and the kernel emits out.T [D, Nq]; the host transposes back.  The
softmax denominator is folded into V row-scaling so the normalize step touches
only 128x128 tiles per key tile.

Phase A (per key tile): scores matmul (fp16 in, fp32 psum) -> exp on ScalarE
(row-sums half fused into the activation, half on VectorE) -> E resident in
SBUF as fp16 -> this key tile's contribution to out.T for query half A
accumulated in PSUM (hides the second GEMM under the ScalarE exp span).
Phase B: query half B accumulated over all 16 key tiles, flushed, DMA'd out.

The ScalarE exp pass (8.4M elements/core) is the roofline.  The first
score matmul only needs the first key tile and first q chunk, so those land
in their own SBUF tiles with their DMAs issued first (the Tile dep tracker
is whole-tile: one big qT tile would stall the first matmul on all four qT
piece DMAs, ~4us of dead startup).  Matmuls are ordered to keep stationary
runs contiguous (explicit ordering deps keep the scheduler from splitting
them); paired A/B loop-diff timing on hardware measured this layout ~20%
faster than the single-tile baseline.
"""

import numpy as np

import concourse.bass as bass
import concourse.mybir as mybir
import concourse.tile as tile
from concourse.bass_utils import run_bass_kernel_spmd
from concourse.tile_rust import add_dep_helper

B, N, D = 4, 4096, 128
P = 128
NK = 2048                 # keys per core (half of 4096)
KT_TILES = NK // P        # 16 key tiles of 128
SCALE = 1.0 / np.sqrt(128.0)

F32 = mybir.dt.float32
F16 = mybir.dt.float16


def emit_body(nc, tc, pools, aps, skip_act=False, skip_phaseb=False, skip_gemm1=False, c2048=False, dve_rowsum=False):
    big, epool, small, spsum, opsum = pools
    qt_d, kt_d, v_d, out_d = aps

    qT0 = big.tile([P, 1024], F16, tag="qT0")       # [d, q 0:1024]
    qTr = big.tile([P, N - 1024], F16, tag="qTr")   # [d, q 1024:4096]
    kT0 = big.tile([P, P], F16, tag="kT0")          # [d, k tile 0]
    kTr = big.tile([P, NK - P], F16, tag="kTr")     # [d, k tiles 1..15]
    vsb = big.tile([P, KT_TILES, D], F16, tag="v")  # [k_in_tile, k_tile, d]
    oacc = big.tile([P, N], F32, tag="oacc")        # [d, q]

    def kslice(kt, lo=0, hi=P):
        if kt == 0:
            return kT0[:, lo:hi]
        return kTr[:, (kt - 1) * P + lo : (kt - 1) * P + hi]

    def qslice(lo, width):
        if lo + width <= 1024:
            return qT0[:, lo : lo + width]
        assert lo >= 1024
        return qTr[:, lo - 1024 : lo - 1024 + width]

    # first key tile + first q chunk land first so the pipeline starts early
    nc.sync.dma_start(kT0[:], kt_d[:, 0:P])
    nc.sync.dma_start(qT0[:], qt_d[:, 0:1024])
    nc.sync.dma_start(kTr[:], kt_d[:, P:NK])
    for _qc in range(1, 4):
        nc.sync.dma_start(
            qTr[:, (_qc - 1) * 1024 : _qc * 1024],
            qt_d[:, _qc * 1024 : (_qc + 1) * 1024],
        )
    nc.sync.dma_start(vsb[:], v_d.rearrange("(t p) d -> p t d", p=P))

    s_free = 2048 if c2048 else 1024
    # Warm-up matmul: first real matmul then carries at most one sync wait.
    Swarm = spsum.tile([P, s_free], F32, tag="S")
    nc.tensor.matmul(
        Swarm[0:1, 0:1], lhsT=kT0[:, 0:1], rhs=qT0[:, 0:1], start=True, stop=True
    )

    e_tiles = []
    v_tiles = []
    # Output accumulators for query half A (cols 0..2047) are built up during
    # phase A so most of the second GEMM hides under the exp (ScalarE) span.
    oa_tiles = []
    if not skip_phaseb and not c2048:
        for _oc in range(2):
            O_a = opsum.tile([P, 1024], F32, tag="O")
            oa_tiles.append(O_a)

    # Phase A: per key tile, scores + exp (row-sum fused) + scaled V,
    # then this key tile's contribution to out.T for query half A.
    for kt in range(KT_TILES):
        E = epool.tile([P, N], F16, tag=f"E{kt}")   # [k, q] = exp(scores.T)
        n_h = N // s_free
        rs = small.tile([P, n_h], F32, tag=f"rs{kt}")
        last_g1 = None
        for h in range(0 if skip_gemm1 else n_h):
            S = spsum.tile([P, s_free], F32, tag="S")
            for u in range(s_free // 512):
                last_g1 = nc.tensor.matmul(
                    S[:, u * 512 : (u + 1) * 512],
                    lhsT=kslice(kt),
                    rhs=qslice(h * s_free + u * 512, 512),
                    start=True,
                    stop=True,
                )
            if not skip_act:
                if dve_rowsum and h < n_h // 2:
                    # row-sum for this chunk computed on VectorE from E
                    # (frees the ScalarE accumulator-read overhead)
                    nc.scalar.activation(
                        out=E[:, h * s_free : (h + 1) * s_free],
                        in_=S[:],
                        func=mybir.ActivationFunctionType.Exp,
                        scale=float(SCALE),
                    )
                    nc.vector.reduce_sum(
                        out=rs[:, h : h + 1],
                        in_=E[:, h * s_free : (h + 1) * s_free],
                        axis=mybir.AxisListType.X,
                    )
                else:
                    nc.scalar.activation(
                        out=E[:, h * s_free : (h + 1) * s_free],
                        in_=S[:],
                        func=mybir.ActivationFunctionType.Exp,
                        scale=float(SCALE),
                        accum_out=rs[:, h : h + 1],
                    )
        rsum = small.tile([P, 1], F32, tag="rsum")
        recip = small.tile([P, 1], F32, tag="recip")
        vsc = small.tile([P, D], F16, tag=f"vsc{kt}")  # [k, d] * r[k]
        if not skip_act:
            nc.vector.reduce_sum(out=rsum[:], in_=rs[:], axis=mybir.AxisListType.X)
            nc.vector.reciprocal(recip[:], rsum[:])
            nc.vector.tensor_scalar_mul(vsc[:], vsb[:, kt, :], recip[:])
        elif not skip_phaseb:
            nc.sync.dma_start(E[:], qt_d[:])  # debug path
            nc.vector.tensor_copy(out=vsc[:], in_=vsb[:, kt, :])
        e_tiles.append(E)
        v_tiles.append(vsc)

        if not skip_phaseb and not c2048:
            # Emit the PREVIOUS key tile's half-A output matmuls here, ordered
            # after this tile's scores matmuls (ordering-only deps).  This
            # keeps each kT weight-load run contiguous: the scheduler would
            # otherwise wedge the vsc burst into the middle of the scores
            # run, costing an extra ~1us weight transition per key tile.
            if kt > 0:
                pv, pe_t, pkt = pending_g2a
                for oc in range(2):
                    for u in range(2):
                        mm = nc.tensor.matmul(
                            oa_tiles[oc][:, u * 512 : (u + 1) * 512],
                            lhsT=pv[:],
                            rhs=pe_t[:, oc * 1024 + u * 512 : oc * 1024 + (u + 1) * 512],
                            start=(pkt == 0),
                            stop=False,
                        )
                        if last_g1 is not None:
                            add_dep_helper(
                                mm.ins,
                                last_g1.ins,
                                sync=False,
                                reason="keep kT weight-load run contiguous",
                            )
            pending_g2a = (vsc, E, kt)

    if not skip_phaseb and not c2048:
        pv, pe_t, pkt = pending_g2a
        for oc in range(2):
            for u in range(2):
                nc.tensor.matmul(
                    oa_tiles[oc][:, u * 512 : (u + 1) * 512],
                    lhsT=pv[:],
                    rhs=pe_t[:, oc * 1024 + u * 512 : oc * 1024 + (u + 1) * 512],
                    start=False,
                    stop=True,
                )

    # Phase B: flush half A, then accumulate query half B (cols 2048..4095).
    if not skip_phaseb and c2048:
        # no interleave: both halves accumulated here, kt-outer.
        # O tiles live in the S pool slots (the opsum pool has no banks
        # left when S is [128, 2048] double-buffered).
        for half in range(2):
            hb_tiles = []
            for _oc in range(2):
                O_h = spsum.tile([P, 1024], F32, tag="S")
                hb_tiles.append(O_h)
            for kt in range(KT_TILES):
                for oc in range(2):
                    for u in range(2):
                        base = half * 2048 + oc * 1024 + u * 512
                        nc.tensor.matmul(
                            hb_tiles[oc][:, u * 512 : (u + 1) * 512],
                            lhsT=v_tiles[kt][:],
                            rhs=e_tiles[kt][:, base : base + 512],
                            start=(kt == 0),
                            stop=(kt == KT_TILES - 1),
                        )
            for oc in range(2):
                lo = half * 2048 + oc * 1024
                nc.vector.tensor_copy(out=oacc[:, lo : lo + 1024], in_=hb_tiles[oc][:])
                nc.sync.dma_start(out_d[:, lo : lo + 1024], oacc[:, lo : lo + 1024])
    elif not skip_phaseb:
        for oc in range(2):
            nc.vector.tensor_copy(
                out=oacc[:, oc * 1024 : (oc + 1) * 1024], in_=oa_tiles[oc][:]
            )
            nc.sync.dma_start(
                out_d[:, oc * 1024 : (oc + 1) * 1024],
                oacc[:, oc * 1024 : (oc + 1) * 1024],
            )
        ob_tiles = []
        for _oc in range(2):
            O_b = opsum.tile([P, 1024], F32, tag="O")
            ob_tiles.append(O_b)
        prev_mm = None
        # oc-major: ob0's accumulation chain completes ~7us before ob1's,
        # so its PSUM->SBUF copy + out DMA overlap the remaining matmuls
        for oc in range(2):
            for kt in range(KT_TILES):
                for u in range(2):
                    mm = nc.tensor.matmul(
                        ob_tiles[oc][:, u * 512 : (u + 1) * 512],
                        lhsT=v_tiles[kt][:],
                        rhs=e_tiles[kt][:, 2048 + oc * 1024 + u * 512 : 2048 + oc * 1024 + (u + 1) * 512],
                        start=(kt == 0),
                        stop=(kt == KT_TILES - 1),
                    )
                    if prev_mm is not None:
                        add_dep_helper(
                            mm.ins, prev_mm.ins, sync=False,
                            reason="serial tail keeps PE dense",
                        )
                    prev_mm = mm
            nc.vector.tensor_copy(
                out=oacc[:, 2048 + oc * 1024 : 2048 + (oc + 1) * 1024],
                in_=ob_tiles[oc][:],
            )
            nc.sync.dma_start(
                out_d[:, 2048 + oc * 1024 : 2048 + (oc + 1) * 1024],
                oacc[:, 2048 + oc * 1024 : 2048 + (oc + 1) * 1024],
            )
    else:
        nc.gpsimd.memset(oacc[:], 0.0)
        nc.sync.dma_start(out_d[:], oacc[:])


def build_bass(repeat=1, skip_act=False, skip_phaseb=False, skip_gemm1=False, loop=False, c2048=False, dve_rowsum=False):
    nc = bass.Bass("TRN2", target_bir_lowering=False, debug=False)
    qt_d = nc.dram_tensor("qt", [P, N], F16, kind="ExternalInput").ap()
    kt_d = nc.dram_tensor("kt", [P, NK], F16, kind="ExternalInput").ap()
    v_d = nc.dram_tensor("v", [NK, D], F16, kind="ExternalInput").ap()
    out_d = nc.dram_tensor("out_t", [P, N], F32, kind="ExternalOutput").ap()

    with tile.TileContext(nc) as tc:
        import contextlib
        with (
            tc.tile_pool(name="big", bufs=1) as big,
            tc.tile_pool(name="epool", bufs=1) as epool,
            tc.tile_pool(name="small", bufs=2) as small,
            tc.tile_pool(name="spsum", bufs=2, space="PSUM") as spsum,
            (contextlib.nullcontext(None) if c2048
             else tc.tile_pool(name="opsum", bufs=2, space="PSUM")) as opsum,
        ):
            def body():
                emit_body(
                    nc,
                    tc,
                    (big, epool, small, spsum, opsum),
                    (qt_d, kt_d, v_d, out_d),
                    skip_act=skip_act,
                    skip_phaseb=skip_phaseb,
                    skip_gemm1=skip_gemm1,
                    c2048=c2048,
                    dve_rowsum=dve_rowsum,
                )

            if loop and repeat > 1:
                with tc.For_i(
                    0, repeat, 1,
                    hint_engines=(mybir.EngineType.PE, mybir.EngineType.Activation),
                ):
                    body()
            else:
                for _ in range(repeat):
                    body()
    return nc


def legalize_waits(nc, max_waits=1):
    """Hoist excess semaphore waits into standalone EventSemaphore ops.

    The walrus codegen for several engine instruction structs accepts only a
    single sync-wait command; Tile sometimes emits more.  Executing the extra
    waits in a preceding same-engine EventSemaphore is semantically identical
    (the engine runs its stream in order).
    """
    for fn in nc.m.functions:
        for blk in fn.blocks:
            out = []
            for inst in blk.instructions:
                si = inst.sync_info
                if (
                    si is not None
                    and si.on_wait
                    and len(si.on_wait) > max_waits
                    and inst.opcode != "EventSemaphore"
                ):
                    waits = list(si.on_wait)
                    extra, keep = waits[:-max_waits], waits[-max_waits:]
                    for n, w in enumerate(extra):
                        out.append(
                            mybir.InstEventSemaphore(
                                name=f"{inst.name}_prewait{n}",
                                engine=inst.engine,
                                ins=[],
                                outs=[],
                                sync_info=mybir.SyncInfo(on_wait=[w], on_update=[]),
                            )
                        )
                    si.on_wait = keep
                out.append(inst)
            blk.instructions = out
    return nc


_NC_CACHE = {}


def _get_nc(repeat=1, **kw):
    key = ("nc", repeat, tuple(sorted(kw.items())))
    if key not in _NC_CACHE:
        _NC_CACHE[key] = legalize_waits(build_bass(repeat, **kw))
    return _NC_CACHE[key]


def kernel(q, k, v):
    q = np.asarray(q, dtype=np.float32)
    k = np.asarray(k, dtype=np.float32)
    v = np.asarray(v, dtype=np.float32)

    in_maps = []
    for c in range(8):
        b, h = c // 2, c % 2
        in_maps.append(
            {
                "qt": np.ascontiguousarray(q[b].T).astype(np.float16),
                "kt": np.ascontiguousarray(k[b, h * NK : (h + 1) * NK].T).astype(np.float16),
                "v": np.ascontiguousarray(v[b, h * NK : (h + 1) * NK]).astype(np.float16),
            }
        )

    nc = _get_nc()
    res = run_bass_kernel_spmd(nc, in_maps, list(range(8))).results

    out = np.empty((B, N, D), dtype=np.float32)
    for b in range(B):
        out[b] = (res[2 * b]["out_t"] + res[2 * b + 1]["out_t"]).T
    return out

